# revision 21
# baseline (speedup 1.0000x reference)
"""Trainium2 Bass kernel for AdditiveUnpoolingWrapper.

  proj_down = gelu(LN(down @ W_down + b_down))          [M, 128]
  proj_skip = gelu(LN(residual @ W_skip + b_skip))      [N, 128]
  out       = proj_skip + proj_down[subbuck_idx]        [N, 128]

Sharding strategy (8 cores, all compute on device):
  The pooled-bucket space M=262144 is split into 8 contiguous ranges of
  32768 rows; core i owns range i and computes that slice of proj_down
  into a 16 MB local DRAM table. Points (rows of residual) are assigned
  to the core that owns their subbuck_idx — i.e. data-parallel over
  points with a bucket-aligned assignment — so the gather is local to
  the core's own table and local indices fit in [0, 32768). The host
  sorts points by subbuck_idx (shards become contiguous, and in-shard
  gathers hit ascending addresses), pads each shard to a common CAP,
  and inverse-permutes the concatenated device outputs back to the
  original point order.  Weights are replicated.
"""

import numpy as np

N = 524288
M = 262144
C_IN = 256
C_SKIP = 128
C_OUT = 128
LN_EPS = 1e-5
NCORES = 8
SH = M // NCORES  # table rows per core (32768)
P = 128
GRP = 4  # 128-point groups per chunk
CHUNK = P * GRP  # points per chunk (512)

_PROG_CACHE = {}


def _wrap_idx_i16(li, cap):
    """dma_gather index layout: index i lives at partition i%16, free i//16,
    replicated across the 8 gpsimd cores (partition blocks of 16)."""
    w = li.astype(np.int16).reshape(cap // 16, 16).T
    return np.ascontiguousarray(np.tile(w, (8, 1)))


def _build_program(cap, dn_rows, trivial_params):
    """Build + compile the SPMD Bass program.

    cap      : padded points per core (multiple of CHUNK)
    dn_rows  : down/table rows per core (multiple of CHUNK)
    trivial_params : True when b_down/b_skip are 0 and ln_g/ln_b are 1/0
                     (lets us skip the per-free-element affine ops).
    """
    from contextlib import ExitStack

    import concourse.bass as bass
    import concourse.tile as tile
    from concourse import bacc, mybir

    from concourse import library_config

    f32 = mybir.dt.float32
    i16 = mybir.dt.int16
    AF = mybir.ActivationFunctionType
    ALU = mybir.AluOpType

    nc = bacc.Bacc("TRN2", target_bir_lowering=False, debug=False,
                   num_devices=NCORES)

    down_t = nc.dram_tensor("down_t", [C_IN, dn_rows], f32, kind="ExternalInput").ap()
    resid_t = nc.dram_tensor("resid_t", [C_SKIP, cap], f32, kind="ExternalInput").ap()
    idxw = nc.dram_tensor("idxw", [P, cap // 16], i16, kind="ExternalInput").ap()
    w_down = nc.dram_tensor("w_down", [C_IN, C_OUT], f32, kind="ExternalInput").ap()
    w_skip = nc.dram_tensor("w_skip", [C_SKIP, C_OUT], f32, kind="ExternalInput").ap()
    # packed per-channel params: [b_down, g_down, bl_down, b_skip, g_skip, bl_skip]
    params = nc.dram_tensor("params", [6, C_OUT], f32, kind="ExternalInput").ap()
    table = nc.dram_tensor("table", [dn_rows, C_OUT], f32, kind="Internal").ap()
    out = nc.dram_tensor("out", [cap, C_OUT], f32, kind="ExternalOutput").ap()

    kd = C_IN // P  # 2 k-chunks for the down projection

    with tile.TileContext(nc) as tc, ExitStack() as ctx:
        consts = ctx.enter_context(tc.tile_pool(name="consts", bufs=1))
        a_in = ctx.enter_context(tc.tile_pool(name="a_in", bufs=3))
        a_out = ctx.enter_context(tc.tile_pool(name="a_out", bufs=3))
        a_psum = ctx.enter_context(tc.tile_pool(name="a_psum", bufs=3, space="PSUM"))
        b_in = ctx.enter_context(tc.tile_pool(name="b_in", bufs=3))
        b_out = ctx.enter_context(tc.tile_pool(name="b_out", bufs=3))
        b_psum = ctx.enter_context(tc.tile_pool(name="b_psum", bufs=3, space="PSUM"))
        stats = ctx.enter_context(tc.tile_pool(name="stats", bufs=6))

        # ---- constants ----
        wd = consts.tile([P, kd, C_OUT], f32, tag="wd")
        nc.sync.dma_start(wd[:], w_down.rearrange("(a p) n -> p a n", p=P))
        ws = consts.tile([P, C_OUT], f32, tag="ws")
        nc.sync.dma_start(ws[:], w_skip[:, :])
        eps_t = consts.tile([P, 1], f32, tag="eps")
        nc.vector.memset(eps_t[:], LN_EPS)
        idx_sb = consts.tile([P, cap // 16], i16, tag="idx")
        nc.sync.dma_start(idx_sb[:], idxw[:, :])
        with tc.tile_critical():
            nc.gpsimd.load_library(library_config.mlp)

        if not trivial_params:
            # broadcast per-channel params across all 128 partitions
            par_sb = consts.tile([P, 6, C_OUT], f32, tag="par")
            par_bcast = bass.AP(
                tensor=params.tensor,
                offset=params.offset,
                ap=[[0, P], params.ap[0], params.ap[1]],
            )
            nc.sync.dma_start(par_sb[:], par_bcast)

        def ln_act_store(psum, bias_idx, g_idx, bl_idx, dest_tile):
            """LayerNorm + gelu from a [P, GRP*128] psum into dest_tile
            (a [P, GRP, 128] SBUF tile)."""
            psum3 = psum[:].rearrange("p (g c) -> p g c", g=GRP)
            if not trivial_params:
                # x += bias (per out-channel, broadcast over partitions)
                nc.vector.tensor_tensor(
                    out=psum3, in0=psum3,
                    in1=par_sb[:, bias_idx:bias_idx + 1, :].to_broadcast(
                        [P, GRP, C_OUT]),
                    op=ALU.add)
            st = stats.tile([P, GRP, 6], f32, tag="bn")
            mv = stats.tile([P, GRP, 2], f32, tag="mv")
            for g in range(GRP):
                nc.vector.bn_stats(st[:, g, :], psum[:, g * C_OUT:(g + 1) * C_OUT])
                nc.vector.bn_aggr(mv[:, g, :], st[:, g, :])
            rstd = stats.tile([P, GRP], f32, tag="rstd")
            nc.scalar.activation(rstd[:], mv[:, :, 1], AF.Sqrt, bias=eps_t[:])
            nc.vector.reciprocal(rstd[:], rstd[:])
            nbias = stats.tile([P, GRP], f32, tag="nbias")
            nc.vector.tensor_tensor(out=nbias[:], in0=mv[:, :, 0], in1=rstd[:],
                                    op=ALU.mult)
            nc.vector.tensor_scalar(out=nbias[:], in0=nbias[:], scalar1=-1.0,
                                    scalar2=None, op0=ALU.mult)
            if trivial_params:
                # gelu((x - mu) * rstd) fused into the activation op
                for g in range(GRP):
                    nc.scalar.activation(
                        dest_tile[:, g, :], psum[:, g * C_OUT:(g + 1) * C_OUT],
                        AF.Gelu_apprx_tanh,
                        bias=nbias[:, g:g + 1], scale=rstd[:, g:g + 1])
            else:
                xn = stats.tile([P, GRP, C_OUT], f32, tag="xn")
                for g in range(GRP):
                    nc.scalar.activation(
                        xn[:, g, :], psum[:, g * C_OUT:(g + 1) * C_OUT],
                        AF.Identity,
                        bias=nbias[:, g:g + 1], scale=rstd[:, g:g + 1])
                nc.vector.tensor_tensor(
                    out=xn[:], in0=xn[:],
                    in1=par_sb[:, g_idx:g_idx + 1, :].to_broadcast([P, GRP, C_OUT]),
                    op=ALU.mult)
                nc.vector.tensor_tensor(
                    out=xn[:], in0=xn[:],
                    in1=par_sb[:, bl_idx:bl_idx + 1, :].to_broadcast([P, GRP, C_OUT]),
                    op=ALU.add)
                nc.scalar.activation(dest_tile[:, :, :], xn[:],
                                     AF.Gelu_apprx_tanh)

        # ---- phase A: build this core's slice of proj_down ----
        from bass_rust import add_dep_helper

        table_writes = []
        down3 = down_t.rearrange("(a p) n -> p a n", p=P)
        with nc.named_scope("phaseA"):
            for c in range(dn_rows // CHUNK):
                o = c * CHUNK
                dtile = a_in.tile([P, kd, CHUNK], f32, tag="dtile")
                nc.sync.dma_start(dtile[:], down3[:, :, o:o + CHUNK])
                psum = a_psum.tile([P, CHUNK], f32, tag="apsum")
                for g in range(GRP):
                    sl = slice(g * P, (g + 1) * P)
                    for a in range(kd):
                        nc.tensor.matmul(out=psum[:, sl], lhsT=dtile[:, a, sl],
                                         rhs=wd[:, a, :],
                                         start=(a == 0), stop=(a == kd - 1))
                ptile = a_out.tile([P, GRP, C_OUT], f32, tag="ptile")
                ln_act_store(psum, 0, 1, 2, ptile)
                w = nc.sync.dma_start(
                    table[o:o + CHUNK, :].rearrange("(g p) c -> p g c", p=P),
                    ptile[:])
                table_writes.append(w)

        # join node: all table writes complete (DRAM RAW deps between DMAs
        # are not tracked automatically, so make the gathers wait explicitly)
        table_ready = nc.sync.nop()
        for w in table_writes:
            add_dep_helper(table_ready.ins, w.ins,
                           reason="table_ready waits on table write")

        # ---- phase B: skip projection + gather + add ----
        with nc.named_scope("phaseB"):
            for c in range(cap // CHUNK):
                o = c * CHUNK
                rtile = b_in.tile([P, CHUNK], f32, tag="rtile")
                nc.sync.dma_start(rtile[:], resid_t[:, o:o + CHUNK])
                psum = b_psum.tile([P, CHUNK], f32, tag="bpsum")
                for g in range(GRP):
                    sl = slice(g * P, (g + 1) * P)
                    nc.tensor.matmul(out=psum[:, sl], lhsT=rtile[:, sl],
                                     rhs=ws[:, :], start=True, stop=True)
                stile = b_out.tile([P, GRP, C_OUT], f32, tag="stile")
                ln_act_store(psum, 3, 4, 5, stile)
                # gather proj_down rows for these points
                gtile = b_out.tile([P, GRP, C_OUT], f32, tag="gtile")
                gi = nc.gpsimd.dma_gather(
                    gtile[:], table[:, :],
                    idx_sb[:, c * (CHUNK // 16):(c + 1) * (CHUNK // 16)],
                    CHUNK, CHUNK, C_OUT)
                add_dep_helper(gi.ins, table_ready.ins,
                               reason="gather waits on table_ready")
                nc.vector.tensor_tensor(out=stile[:], in0=stile[:],
                                        in1=gtile[:], op=ALU.add)
                nc.sync.dma_start(
                    out[o:o + CHUNK, :].rearrange("(g p) c -> p g c", p=P),
                    stile[:])

    nc.compile()
    return nc


def _get_program(cap, dn_rows, trivial_params):
    key = (cap, dn_rows, trivial_params)
    if key not in _PROG_CACHE:
        _PROG_CACHE[key] = _build_program(cap, dn_rows, trivial_params)
    return _PROG_CACHE[key]


def kernel(residual, down, W_down, b_down, ln_g_down, ln_b_down,
           W_skip, b_skip, ln_g_skip, ln_b_skip, subbuck_idx):
    from concourse.bass_utils import run_bass_kernel_spmd

    residual = np.ascontiguousarray(np.asarray(residual, dtype=np.float32))
    down = np.ascontiguousarray(np.asarray(down, dtype=np.float32))
    W_down = np.ascontiguousarray(np.asarray(W_down, dtype=np.float32))
    W_skip = np.ascontiguousarray(np.asarray(W_skip, dtype=np.float32))
    idx = np.asarray(subbuck_idx).astype(np.int32)
    pvecs = [np.asarray(v, dtype=np.float32) for v in
             (b_down, ln_g_down, ln_b_down, b_skip, ln_g_skip, ln_b_skip)]
    trivial = (not pvecs[0].any() and not pvecs[3].any()
               and np.all(pvecs[1] == 1) and np.all(pvecs[4] == 1)
               and not pvecs[2].any() and not pvecs[5].any())
    params = np.stack(pvecs).astype(np.float32)

    n = idx.shape[0]
    assert residual.shape == (n, C_SKIP) and down.shape == (M, C_IN)

    # ---- host-side sharding: sort points by bucket ----
    order = np.argsort(idx, kind="stable")
    sorted_idx = idx[order]
    bounds = np.searchsorted(sorted_idx, np.arange(NCORES + 1) * SH)
    counts = np.diff(bounds)
    cap = int(np.ceil(max(counts.max(), 1) / CHUNK) * CHUNK)

    nc = _get_program(cap, SH, trivial)

    down_T = np.ascontiguousarray(down.T)  # [C_IN, M]
    in_maps = []
    segs = []
    for i in range(NCORES):
        seg = order[bounds[i]:bounds[i + 1]]
        segs.append(seg)
        ni = seg.shape[0]
        rt = np.zeros((cap, C_SKIP), np.float32)
        rt[:ni] = residual[seg]
        li = np.zeros(cap, np.int32)
        li[:ni] = sorted_idx[bounds[i]:bounds[i + 1]] - i * SH
        in_maps.append({
            "down_t": np.ascontiguousarray(down_T[:, i * SH:(i + 1) * SH]),
            "resid_t": np.ascontiguousarray(rt.T),
            "idxw": _wrap_idx_i16(li, cap),
            "w_down": W_down,
            "w_skip": W_skip,
            "params": params,
        })

    global _LAST_RUN
    _LAST_RUN = (nc, in_maps)
    res = run_bass_kernel_spmd(nc, in_maps, core_ids=list(range(NCORES)))

    out = np.empty((n, C_OUT), np.float32)
    for i in range(NCORES):
        out[segs[i]] = res.results[i]["out"][:segs[i].shape[0]]
    return out
